# revision 13
# baseline (speedup 1.0000x reference)
"""Trainium2 Bass kernel for causal ("FORWARD" direction) multi-head attention.

Reference computation (per batch b, n_heads=8, d=128):
  Q = x @ Wq.T ; K = x @ Wk.T ; V = x @ Wv.T          (nn.Linear, no bias)
  scores[h,i,j] = (Qh[i] . Kh[j]) / sqrt(d)
  scores += -10000 where j <= i   (keeps strict upper triangle j > i)
  attn = softmax(scores, axis=j) ; out = attn @ Vh ; concat heads
  Row i=1023 is fully masked; jax softmax's max-subtraction makes it equal
  softmax of the *raw* scores, so the kernel keeps column i=1023 unmasked.

Sharding: data-parallel over batch B=8 -> 8 cores, no collectives.

Device layout (per core, everything transposed so the softmax reduction is a
matmul-friendly partition-dim reduction):
  xT[k,t]       : x.T                                  [1024,1024]
  qT/kT[o,t]    : per head-group of 4 heads            via Wq.T/Wk.T as lhsT
  v[t,o]        : natural V                            via xT as lhsT
  S_T[j,i]      = kT_tile.T @ qT  (contraction over d=128, single tile)
  expS          = exp(S_T + adder)   (adder patterns precomputed on host)
  U_T[dd,i]     = sum_j V[j,dd] expS[j,i]   (matmul accum over j tiles)
  colsum[*,i]   = ones.T @ expS             (partition-broadcast row of sums)
  out_T         = U_T * reciprocal(colsum); PE-transpose -> out[i,dd] -> DRAM
"""

import os
import sys

import numpy as np

if "/opt/trn_rl_repo" not in sys.path:
    sys.path.insert(0, "/opt/trn_rl_repo")

B, T, D, H, DH = 8, 1024, 1024, 8, 128
P = 128          # partition tile
NI = 512         # i-chunk (moving free size)
NG, GH = 2, 4    # head groups x heads per group
NKT = T // P     # 8 contraction tiles
USE_F32R = os.environ.get("KERNEL_MM_DT", "f32r") == "f32r"

_PROGRAM = None  # cached compiled Bass program


def _adder_patterns() -> np.ndarray:
    """[128, 8*512] f32. Blocks 0..3: adders for tiles (jt, ic=0); blocks
    4..7: adders for tiles (jt, ic=1) with column i=1023 left unmasked."""
    ad = np.zeros((P, 8, NI), np.float32)
    j = np.arange(P)
    i = np.arange(NI)
    for jt in range(4):
        ad[:, jt, :] = np.where((P * jt + j)[:, None] <= i[None, :], -10000.0, 0.0)
    for jt in range(4, 8):
        blk = np.where((P * jt + j)[:, None] <= (NI + i)[None, :], -10000.0, 0.0)
        blk[:, NI - 1] = 0.0  # column i=1023 stays raw
        ad[:, jt, :] = blk
    return np.ascontiguousarray(ad.reshape(P, 8 * NI))


def build_program(use_f32r: bool = USE_F32R, compile: bool = True):
    import concourse.bass as bass  # noqa: F401
    import concourse.tile as tile
    from concourse import bacc, mybir

    f32 = mybir.dt.float32
    mdt = mybir.dt.float32r if use_f32r else mybir.dt.float32
    Exp = mybir.ActivationFunctionType.Exp
    Copy = mybir.ActivationFunctionType.Copy
    ADD = mybir.AluOpType.add
    MUL = mybir.AluOpType.mult

    nc = bacc.Bacc(
        "TRN2",
        target_bir_lowering=False,
        debug=False,
        enable_asserts=False,
        num_devices=B,
    )

    xT_d = nc.dram_tensor("xT", [D, T], mdt, kind="ExternalInput")
    wq_d = nc.dram_tensor("wqT", [D, D], mdt, kind="ExternalInput")
    wk_d = nc.dram_tensor("wkT", [D, D], mdt, kind="ExternalInput")
    wv_d = nc.dram_tensor("wvT", [D, D], mdt, kind="ExternalInput")
    ad_d = nc.dram_tensor("adders", [P, 8 * NI], f32, kind="ExternalInput")
    on_d = nc.dram_tensor("ones_t", [P, P], mdt, kind="ExternalInput")
    id_d = nc.dram_tensor("ident", [P, P], f32, kind="ExternalInput")
    out_d = nc.dram_tensor("out", [T, D], f32, kind="ExternalOutput")

    with tile.TileContext(nc) as tc:
        with (
            tc.tile_pool(name="sb", bufs=1) as sb,
            tc.tile_pool(name="ps", bufs=1, space="PSUM") as ps,
        ):
            # ---------------- resident loads ----------------
            xT = []
            for k in range(NKT):
                t = sb.tile([P, T], mdt, tag=f"xT{k}", name=f"xT{k}")
                nc.sync.dma_start(t[:], xT_d.ap()[P * k : P * (k + 1), :])
                xT.append(t)
            adders = sb.tile([P, 8 * NI], f32, tag="adders", name="adders")
            nc.gpsimd.dma_start(adders[:], ad_d.ap()[:])
            ones = sb.tile([P, P], mdt, tag="ones", name="ones")
            nc.gpsimd.dma_start(ones[:], on_d.ap()[:])
            ident = sb.tile([P, P], f32, tag="ident", name="ident")
            nc.gpsimd.dma_start(ident[:], id_d.ap()[:])

            copy_flip = [0]

            def psum_to_sbuf(dst_ap, src_ap):
                # alternate ACT / DVE to balance engine load
                if copy_flip[0] % 2 == 0:
                    nc.scalar.activation(dst_ap, src_ap, Copy)
                else:
                    nc.vector.tensor_copy(dst_ap, src_ap)
                copy_flip[0] += 1

            for g in range(NG):
                # ---------------- group weight loads ----------------
                wq_g, wk_g, wv_g = [], [], []
                for kt in range(NKT):
                    for nm, lst, dram in (
                        ("wq", wq_g, wq_d),
                        ("wk", wk_g, wk_d),
                        ("wv", wv_g, wv_d),
                    ):
                        w = sb.tile([P, NI], mdt, tag=f"{nm}{kt}", name=f"{nm}{kt}g{g}")
                        nc.scalar.dma_start(
                            w[:],
                            dram.ap()[P * kt : P * (kt + 1), NI * g : NI * (g + 1)],
                        )
                        lst.append(w)

                qT_g = [
                    sb.tile([P, T], mdt, tag=f"qT{ot}", name=f"qT{ot}g{g}")
                    for ot in range(GH)
                ]
                kT_g = [
                    sb.tile([P, T], mdt, tag=f"kT{ot}", name=f"kT{ot}g{g}")
                    for ot in range(GH)
                ]
                v_g = [
                    sb.tile([P, NI], mdt, tag=f"v{tt}", name=f"v{tt}g{g}")
                    for tt in range(NKT)
                ]

                # ---------------- projections ----------------
                def proj_qk(ot):
                    for wlist, dst in ((wq_g, qT_g[ot]), (wk_g, kT_g[ot])):
                        for tci in range(2):
                            pp = ps.tile([P, NI], f32, tag="s", bufs=3, name="pp")
                            for kt in range(NKT):
                                nc.tensor.matmul(
                                    pp[:],
                                    wlist[kt][:, P * ot : P * (ot + 1)],
                                    xT[kt][:, NI * tci : NI * (tci + 1)],
                                    start=(kt == 0),
                                    stop=(kt == NKT - 1),
                                )
                            psum_to_sbuf(dst[:, NI * tci : NI * (tci + 1)], pp[:])

                def proj_v(tt):
                    pp = ps.tile([P, NI], f32, tag="s", bufs=3, name="pp")
                    for kt in range(NKT):
                        nc.tensor.matmul(
                            pp[:],
                            xT[kt][:, P * tt : P * (tt + 1)],
                            wv_g[kt][:],
                            start=(kt == 0),
                            stop=(kt == NKT - 1),
                        )
                    psum_to_sbuf(v_g[tt][:], pp[:])

                proj_qk(0)
                for tt in range(NKT):
                    proj_v(tt)
                for ot in range(1, GH):
                    proj_qk(ot)

                # ---------------- attention ----------------
                for ot in range(GH):
                    h = GH * g + ot
                    qh, kh = qT_g[ot], kT_g[ot]
                    for ic in range(2):
                        jts = list(range(8)) if ic == 0 else [4, 5, 6, 7]
                        nj = len(jts)

                        u_ps = ps.tile([P, NI], f32, tag="u", bufs=2, name="u_ps")
                        c_ps = ps.tile([P, NI], f32, tag="c", bufs=2, name="c_ps")

                        # HW rejects fp32r matmuls with tiny output free size
                        # (s3d3_mm_fp32r_restrictions); run those as plain fp32.
                        def smallmm(ap):
                            return ap.bitcast(f32) if use_f32r else ap

                        col_ps = colE = None
                        if ic == 1:
                            # raw scores for column i=1023, rows j in [0,512)
                            col_ps = ps.tile([P, 8], f32, tag="col", bufs=1, name="col_ps")
                            for jc in range(4):
                                nc.tensor.matmul(
                                    col_ps[:, jc : jc + 1],
                                    smallmm(kh[:, P * jc : P * (jc + 1)]),
                                    smallmm(qh[:, T - 1 : T]),
                                    start=True,
                                    stop=True,
                                )
                            colE = sb.tile([P, 8], mdt, tag="colE", bufs=2, name="colE")
                            nc.scalar.activation(colE[:, 0:4], col_ps[:, 0:4], Exp)

                        pend = []

                        def drain_one():
                            idx, jt, e_sb = pend.pop(0)
                            first, last = idx == 0, idx == nj - 1
                            nc.tensor.matmul(
                                u_ps[:],
                                v_g[jt][:, P * ot : P * (ot + 1)],
                                e_sb[:],
                                start=first,
                                stop=last,
                            )
                            nc.tensor.matmul(
                                c_ps[:],
                                ones[:],
                                e_sb[:],
                                start=first,
                                stop=last,
                            )

                        for idx, jt in enumerate(jts):
                            s_ps = ps.tile([P, NI], f32, tag="s", bufs=3, name="s_ps")
                            nc.tensor.matmul(
                                s_ps[:],
                                kh[:, P * jt : P * (jt + 1)],
                                qh[:, NI * ic : NI * (ic + 1)],
                                start=True,
                                stop=True,
                            )
                            crossing = (ic == 0 and jt < 4) or (ic == 1 and jt >= 4)
                            if crossing:
                                nc.vector.tensor_tensor(
                                    s_ps[:], s_ps[:], adders[:, NI * jt : NI * (jt + 1)], ADD
                                )
                            e_sb = sb.tile([P, NI], mdt, tag="e", bufs=4, name="e_sb")
                            nc.scalar.activation(e_sb[:], s_ps[:], Exp)
                            pend.append((idx, jt, e_sb))
                            while len(pend) > 2:
                                drain_one()
                        while pend:
                            drain_one()

                        if ic == 1:
                            # fold the j<512 contributions of column i=1023 in
                            for jc in range(4):
                                nc.tensor.matmul(
                                    col_ps[:, 4:5],
                                    smallmm(v_g[jc][:, P * ot : P * (ot + 1)]),
                                    smallmm(colE[:, jc : jc + 1]),
                                    start=(jc == 0),
                                    stop=(jc == 3),
                                )
                            for jc in range(4):
                                nc.tensor.matmul(
                                    col_ps[:, 5:6],
                                    smallmm(ones[:]),
                                    smallmm(colE[:, jc : jc + 1]),
                                    start=(jc == 0),
                                    stop=(jc == 3),
                                )
                            colsb = sb.tile([P, 2], f32, tag="colsb", bufs=2, name="colsb")
                            nc.scalar.activation(colsb[:], col_ps[:, 4:6], Copy)
                            nc.vector.tensor_tensor(
                                u_ps[:, NI - 1 : NI], u_ps[:, NI - 1 : NI], colsb[:, 0:1], ADD
                            )
                            nc.vector.tensor_tensor(
                                c_ps[:, NI - 1 : NI], c_ps[:, NI - 1 : NI], colsb[:, 1:2], ADD
                            )

                        recip = sb.tile([P, NI], f32, tag="recip", bufs=2, name="recip")
                        nc.vector.reciprocal(recip[:], c_ps[:])
                        o_sb = sb.tile([P, NI], f32, tag="o", bufs=2, name="o_sb")
                        nc.vector.tensor_tensor(o_sb[:], u_ps[:], recip[:], MUL)

                        tr_ps = ps.tile([P, NI], f32, tag="s", bufs=3, name="tr_ps")
                        for k2 in range(4):
                            nc.tensor.transpose(
                                tr_ps[:, P * k2 : P * (k2 + 1)],
                                o_sb[:, P * k2 : P * (k2 + 1)],
                                ident[:],
                            )
                        fo = sb.tile([P, NI], f32, tag="fo", bufs=3, name="fo")
                        nc.scalar.activation(fo[:], tr_ps[:], Copy)
                        for k2 in range(4):
                            r0 = NI * ic + P * k2
                            nc.gpsimd.dma_start(
                                out_d.ap()[r0 : r0 + P, P * h : P * (h + 1)],
                                fo[:, P * k2 : P * (k2 + 1)],
                            )

    if compile:
        nc.compile()
    return nc


def _get_program():
    global _PROGRAM
    if _PROGRAM is None:
        _PROGRAM = build_program()
    return _PROGRAM


def make_in_maps(x, Wq, Wk, Wv):
    scale = 1.0 / np.sqrt(np.float32(DH))
    wqT = np.ascontiguousarray(np.asarray(Wq, np.float32).T * scale)
    wkT = np.ascontiguousarray(np.asarray(Wk, np.float32).T)
    wvT = np.ascontiguousarray(np.asarray(Wv, np.float32).T)
    adders = _adder_patterns()
    ones = np.ones((P, P), np.float32)
    ident = np.eye(P, dtype=np.float32)
    x = np.asarray(x, np.float32)
    in_maps = []
    for b in range(B):
        in_maps.append(
            {
                "xT": np.ascontiguousarray(x[b].T),
                "wqT": wqT,
                "wkT": wkT,
                "wvT": wvT,
                "adders": adders,
                "ones_t": ones,
                "ident": ident,
            }
        )
    return in_maps


def kernel(x, mask, Wq, Wk, Wv, _trace=False):
    from concourse.bass_utils import run_bass_kernel_spmd

    nc = _get_program()
    in_maps = make_in_maps(x, Wq, Wk, Wv)
    res = run_bass_kernel_spmd(nc, in_maps, core_ids=list(range(B)), trace=_trace)
    out = np.stack([res.results[b]["out"] for b in range(B)], axis=0)
    out = out * np.asarray(mask, np.float32)[:, :, None]
    out = np.ascontiguousarray(out, np.float32)
    if _trace:
        kernel.last_results = res
    return out
